# revision 22
# baseline (speedup 1.0000x reference)
"""Trainium2 Bass kernel for the channel-attention module (v2, restructured).

Reference computation (B=16, N=4096, C=384, H=8, D=48):
    x_in = x @ conv_w.T + conv_b                      # 1x1 conv == linear
    q    = (x_in @ wq.T + bq)  -> [B,H,D,N]
    k, v = (x_in @ wkv.T + bkv) -> 2x [B,H,D,N]
    attn = softmax((q * N**-0.5) @ k^T, axis=-1)      # [B,H,D,D] (over N!)
    out  = attn @ v                                   # [B,H,D,N]
    out  = out.transpose(0,2,1,3).reshape(B,N,C)      # verbatim torch layout
    y    = out @ wp.T + bp

Key restructure vs v1: the v-projection GEMM is eliminated.  Since
out = attn @ (Wv X + bv), fold the block-diagonal attention matrix into
the value weights FIRST:  Wav = [Wv | bv]^T bd  (a tiny [C+1, C]
product), then Z = Wav^T [X; 1] is ONE full-size GEMM instead of two
(v-projection + attn@v).  Z is computed in fp8 DoubleRow (measured: DR
streams at 1 col/cycle like bf16 but contracts 256 rows per pass, so
2 passes replace bf16's 3 -- a 1.5x win); X^T is staged fp8
pair-packed by the host; Wav is cast to fp8 on-chip (ACT cast is RN,
bit-exact with ml_dtypes).  The bv bias rides in the contraction as a
ones-row (host) x bvd-row (on-chip) rank-1 term.

The critical layout trick: stage6 consumes PERMUTED output token
blocks (tau-blocks, t' = 32u + (11*tau)%32, inverted on the host), so
at can stay CHUNK-major (each stage5' PSUM tile dumps contiguously)
while stage6's stationary reads are still contiguous single-stride
slices (the stationary AP allows only one free dimension).  Strided
ACT/DVE copies (64B-hop dst) measure ~5x slower than contiguous and
previously paced the whole kernel, idling the PE long enough to
re-trigger the HAM clock throttle (1.2 GHz) on top.

Per batch (2 per core, data-parallel over B across 8 cores):
  A) Gram G' = [X|1]^T[X|1] in fp8e4 DoubleRow (unchanged from v1),
  B) S-chain from G' (T''=G'wq_ext, S^T per head, exp/softmax) ->
     attn^T -> block-diag bd with column order m = 8d+h (strided ACT
     copies place head h at columns h::8),
  C) Wav fold (tiny GEMM + bias row) -> wav8 fp8,
  D) stage5': at[tok, m] = [X;1]^T Wav, 2 DoubleRow passes per
     128-token chunk; PSUM scattered into at[p, jc, r] (3r+jc = 32m+tc)
     so stage6 reads clean 128-column blocks,
  E) stage6: y = at-blocks @ wp(/256 pre-scaled) + bp, bf16 (fp8 here
     would break the 2e-2 error budget: each fp8 operand on the value
     path costs ~1.1e-2 rel err).

Schedule: batch pipelined -- Gram b1 runs between batch 0's S-chain
stages, stage5'(0) interleaves with S-chain(1), stage6(0) interleaves
with stage5'(1).  Writebacks stream on HWDGE queues; the final 8 blocks
go as split single chunks so the tail drains with the compute.
"""

import sys
import types
from contextlib import ExitStack

import numpy as np

import concourse.bass as bass
import concourse.tile as tile
from concourse import bacc, mybir
from concourse.bass_utils import run_bass_kernel_spmd
from concourse.masks import make_identity

B, N, C, H, D = 16, 4096, 384, 8, 48
N_CORES = 8
BPC = B // N_CORES          # batches per core
NW = 512                    # token window
NW2 = 528                   # xt8 padded row (avoid power-of-2 SBUF strides)
WVP = 400                   # wav8 padded row
NWIN = N // NW              # 8 windows
NCHUNK = N // 128           # 32 token chunks of 128
XNP = 400                   # xn row pad (>= C+1, 16B-aligned for dual-fp8 LDW)
SCALE = float(N) ** -0.5    # 1/64
SCL = 256.0                 # fp8 scale for Wav (wp is pre-divided on host)
F32 = mybir.dt.float32
BF16 = mybir.dt.bfloat16
F8 = mybir.dt.float8e4

# block-diag scatter: (kc, h, off) with off = 48h - 128kc; Sh[dj, c_p] = 1
# iff c_p == dj + off places head h's attn^T rows into bd tile kc.
SCATTER = [
    (0, 0, 0), (0, 1, 48), (0, 2, 96),
    (1, 2, -32), (1, 3, 16), (1, 4, 64), (1, 5, 112),
    (2, 5, -16), (2, 6, 32), (2, 7, 80),
]


def _install_ntff_hook():
    """The agent image's antenv lacks axon_hooks, so trn_boot's NTFF hook
    registration degrades silently and trace=True would crash.  Recreate the
    module and register the ctypes hook so profiling works."""
    try:
        import antenv

        if "antenv.axon_hooks" in sys.modules:
            return
        mod = types.ModuleType("antenv.axon_hooks")
        mod._hook = None
        mod.set_axon_ntff_profile_hook = lambda h: setattr(mod, "_hook", h)
        mod.get_axon_ntff_profile_hook = lambda: mod._hook
        sys.modules["antenv.axon_hooks"] = mod
        antenv.axon_hooks = mod
        from trn_agent_boot.trn_boot import _ntff_profile_via_ctypes

        mod.set_axon_ntff_profile_hook(
            _ntff_profile_via_ctypes("/opt/axon/libaxon_pjrt.so")
        )
    except Exception:
        pass


def build():
    nc = bacc.Bacc("TRN2", target_bir_lowering=False, debug=False,
                   num_devices=N_CORES)

    xn_p = nc.declare_dram_parameter("xn", [BPC, 128, NCHUNK, XNP], F8,
                                     isOutput=False)
    xt_p = nc.declare_dram_parameter("xt8", [BPC, 128, NWIN, 2, 2, NW2], F8,
                                     isOutput=False)
    wq_p = nc.declare_dram_parameter("wqT", [C, C], BF16, isOutput=False)
    wk_p = nc.declare_dram_parameter("wkT", [C, C], BF16, isOutput=False)
    wv_p = nc.declare_dram_parameter("wv2", [C, C], BF16, isOutput=False)
    wp_p = nc.declare_dram_parameter("wpT", [C, C], BF16, isOutput=False)
    bq_p = nc.declare_dram_parameter("bq", [1, C], BF16, isOutput=False)
    bk_p = nc.declare_dram_parameter("bk", [1, C], BF16, isOutput=False)
    bqn_p = nc.declare_dram_parameter("bqN", [1, C], F32, isOutput=False)
    bv_p = nc.declare_dram_parameter("bv2", [C], BF16, isOutput=False)
    bp_p = nc.declare_dram_parameter("bp", [C], F32, isOutput=False)
    outp = nc.declare_dram_parameter("out", [BPC, NCHUNK, 128, C], BF16,
                                    isOutput=True)
    scr_p = nc.declare_dram_parameter("scr", [128, 1], F32, isOutput=True)

    with tile.TileContext(nc) as tc, ExitStack() as ctx:
        const = ctx.enter_context(tc.tile_pool(name="const", bufs=1))
        xnr = ctx.enter_context(tc.tile_pool(name="xnr", bufs=2))
        xtp = ctx.enter_context(tc.tile_pool(name="xtp", bufs=2))
        atp = ctx.enter_context(tc.tile_pool(name="atp", bufs=2))
        sm = ctx.enter_context(tc.tile_pool(name="sm", bufs=2))
        w8 = ctx.enter_context(tc.tile_pool(name="w8", bufs=2))
        yo = ctx.enter_context(tc.tile_pool(name="yo", bufs=6))
        ps_g = ctx.enter_context(tc.tile_pool(name="ps_g", bufs=3, space="PSUM"))
        ps_c = ctx.enter_context(tc.tile_pool(name="ps_c", bufs=3, space="PSUM"))
        ps_v = ctx.enter_context(tc.tile_pool(name="ps_v", bufs=2, space="PSUM"))

        # ---- PE warmup tile first (gpsimd memset; no DMA dependency) ------
        warm = const.tile([128, NW], BF16)
        nc.gpsimd.memset(warm[:], 0.0)

        # ---- input streams -------------------------------------------------
        # Gram input (token-partition fp8), whole batches resident; the first
        # window of batch 0 goes first in small pieces so the Gram isn't
        # gated on the whole prefetch backlog.
        xn_sb = [xnr.tile([128, NCHUNK, XNP], F8, tag="xn", name=f"xn{b}")
                 for b in range(BPC)]
        # batch 0's Gram input split across both HWDGE queues, window-paired
        for lo in range(0, NCHUNK, 4):
            nc.sync.dma_start(xn_sb[0][:, lo:lo + 2, :],
                              xn_p.ap()[0][:, lo:lo + 2, :])
            nc.scalar.dma_start(xn_sb[0][:, lo + 2:lo + 4, :],
                                xn_p.ap()[0][:, lo + 2:lo + 4, :])
        for lo in range(0, NCHUNK, 8):
            nc.sync.dma_start(xn_sb[1][:, lo:lo + 8, :],
                              xn_p.ap()[1][:, lo:lo + 8, :])

        # weights on the ACT HWDGE queue (needed by the S-chain / Wav fold)
        def load_w(param, dtype):
            t = const.tile([128, 3, C], dtype, tag=f"w_{param.name}")
            nc.scalar.dma_start(
                t[:], param.ap().rearrange("(kc p) o -> p kc o", p=128))
            return t

        wq_sb = load_w(wq_p, BF16)
        wk_sb = load_w(wk_p, BF16)
        wv2_sb = load_w(wv_p, BF16)
        bq_row = const.tile([1, C], BF16)
        nc.scalar.dma_start(bq_row[:], bq_p.ap()[:, :])
        bk_row = const.tile([1, C], BF16)
        nc.scalar.dma_start(bk_row[:], bk_p.ap()[:, :])
        bqn_row = const.tile([1, C], F32)
        nc.scalar.dma_start(bqn_row[:], bqn_p.ap()[:, :])
        bv2_sb = const.tile([128, 3], BF16)
        nc.scalar.dma_start(bv2_sb[:],
                            bv_p.ap().rearrange("(kc p) -> p kc", p=128))

        # stage5' input (channel-partition pair-packed fp8), per batch
        # stage5' input: batch 0 on the ACT HWDGE queue, batch 1 on the
        # gpsimd software DGE so neither blocks the Gram streams.
        xt_sb = [xtp.tile([128, NWIN, 2, 2, NW2], F8, tag="xt", name=f"xt{b}")
                 for b in range(BPC)]
        for wlo in range(0, NWIN, 2):
            nc.scalar.dma_start(xt_sb[0][:, wlo:wlo + 2],
                                xt_p.ap()[0][:, wlo:wlo + 2])
        wp_sb = load_w(wp_p, BF16)
        bp_ap = bp_p.ap()
        bp_bc = const.tile([128, C], F32)
        nc.scalar.dma_start(bp_bc[:], bass.AP(
            tensor=bp_ap.tensor, offset=bp_ap.offset,
            ap=[[0, 128], *bp_ap.ap]))
        for wlo in range(0, NWIN, 4):
            nc.gpsimd.dma_start(xt_sb[1][:, wlo:wlo + 4],
                                xt_p.ap()[1][:, wlo:wlo + 4])

        # ---- small constants built on gpsimd (no DMA) ---------------------
        sh_sb = const.tile([48, len(SCATTER), 128], BF16, tag="sh")
        nc.gpsimd.memset(sh_sb[:], 0.0)
        for i, (kc, h, off) in enumerate(SCATTER):
            nc.gpsimd.affine_select(
                out=sh_sb[:, i, :], in_=sh_sb[:, i, :],
                compare_op=mybir.AluOpType.not_equal,
                fill=1.0, base=off, pattern=[[-1, 128]], channel_multiplier=1,
            )
        id128 = const.tile([128, 128], BF16)
        make_identity(nc, id128[:])
        ones48 = const.tile([48, 1], BF16)
        nc.gpsimd.memset(ones48[:], 1.0)
        ones_r = const.tile([1, 48], F32)
        nc.gpsimd.memset(ones_r[:], 1.0)
        ones_row = const.tile([1, 128], BF16)
        nc.gpsimd.memset(ones_row[:], 1.0)

        # warmup matmuls ramp the PE p-state while the first DMAs land
        warm_ps = ps_c.tile([128, NW], F32, tag="psc", name="warm_ps")
        for i in range(10):
            nc.tensor.matmul(warm_ps[:, 0:256], warm[:, 0:128], warm[:, 0:256],
                             start=(i == 0), stop=(i == 9))
        warm_sb = const.tile([128, 1], F32)
        nc.vector.tensor_copy(warm_sb[:], warm_ps[:, 0:1])
        nc.scalar.dma_start(scr_p.ap()[:, :], warm_sb[:])
        exp_warm = const.tile([1, 1], F32)
        nc.scalar.activation(exp_warm[:], warm_ps[0:1, 0:1],
                             mybir.ActivationFunctionType.Exp,
                             bias=0.0, scale=1.0)

        # ---- per-batch state ----------------------------------------------
        st = [dict() for _ in range(BPC)]

        def keep_alive(n):
            # dependency-free dummies: fill PE-idle windows so the HAM
            # activity monitor never re-throttles the clock to 1.2 GHz
            for _ in range(n):
                nc.tensor.matmul(warm_ps[:, 0:256], warm[:, 0:128],
                                 warm[:, 0:256], start=True, stop=True)

        def g_part(b, w):
            # Gram in fp8 DoubleRow: 2 token-chunks (256 contraction) per mm.
            s = st[b]
            if w == 0:
                s["g_ps"] = [ps_g.tile([128, C + 1], F32, tag="g",
                                       name=f"g{b}_{i}") for i in range(3)]
            xn = xn_sb[b]
            for sp in range(2):
                tp = 2 * w + sp
                ksl = slice(4 * w + 2 * sp, 4 * w + 2 * sp + 2)
                for oc in range(3):
                    nc.tensor.matmul(
                        s["g_ps"][oc][:],
                        xn[:, ksl, oc * 128:(oc + 1) * 128],
                        xn[:, ksl, 0:C + 1],
                        start=(tp == 0), stop=(tp == 2 * NWIN - 1),
                        perf_mode=mybir.MatmulPerfMode.DoubleRow,
                    )

        # ---- S-chain stages.  Batch 1's transient PSUM tiles use the Gram
        # pool (its banks are free by then); batch 0's use ps_c.
        def bpool(b):
            return ps_g if b == 1 else ps_c

        def btag(b):
            return "g" if b == 1 else "psc"

        def b_s1_act(b):
            s = st[b]
            g_sb = sm.tile([128, 3, C + 1], BF16, tag="g_sb", name=f"g_sb{b}")
            s["g_sb"] = g_sb
            for oc in range(2):
                nc.scalar.activation(
                    g_sb[:, oc, :], s["g_ps"][oc][:],
                    mybir.ActivationFunctionType.Identity, bias=0.0, scale=1.0)
            nc.vector.tensor_copy(g_sb[:, 2, :], s["g_ps"][2][:])

        def b_s1_pe(b):
            s = st[b]
            g_sb = s["g_sb"]
            # s^T row via identity matmul (s lives in G's ones column)
            st_ps = bpool(b).tile([1, C], F32, tag=btag(b), name="st_ps")
            for kc in range(3):
                nc.tensor.matmul(st_ps[:, kc * 128:(kc + 1) * 128],
                                 g_sb[:, kc, C:C + 1], id128[:],
                                 start=True, stop=True)
            st_row = sm.tile([1, C], BF16, tag="st_row", name="st_row")
            nc.vector.tensor_copy(st_row[:], st_ps[:])
            # T'' row C = wq^T s + N bq
            tr_ps = bpool(b).tile([1, C], F32, tag=btag(b), name="tr_ps")
            for kc in range(3):
                nc.tensor.matmul(tr_ps[:], g_sb[:, kc, C:C + 1],
                                 wq_sb[:, kc, :],
                                 start=(kc == 0), stop=(kc == 2))
            t_row = sm.tile([1, C], BF16, tag="t_row", name="t_row")
            nc.vector.tensor_add(t_row[:], tr_ps[:], bqn_row[:])
            s["st_row"], s["t_row"] = st_row, t_row

        def b_s2(b):
            # T'' = G' wq_ext, c1 blocks (G symmetry: lhsT from G rows)
            s = st[b]
            g_sb = s["g_sb"]
            t_sb = sm.tile([128, 3, C], BF16, tag="t_sb", name=f"t_sb{b}")
            s["t_sb"] = t_sb
            for c1 in range(3):
                t_ps = bpool(b).tile([128, C], F32, tag=btag(b), name="t_ps")
                for kc2 in range(3):
                    nc.tensor.matmul(
                        t_ps[:],
                        g_sb[:, kc2, c1 * 128:(c1 + 1) * 128],
                        wq_sb[:, kc2, :],
                        start=(kc2 == 0), stop=False)
                nc.tensor.matmul(
                    t_ps[:], s["st_row"][:, c1 * 128:(c1 + 1) * 128],
                    bq_row[:], start=False, stop=True)
                if c1 < 2:
                    nc.scalar.activation(
                        t_sb[:, c1, :], t_ps[:],
                        mybir.ActivationFunctionType.Identity,
                        bias=0.0, scale=1.0)
                else:
                    nc.vector.tensor_copy(t_sb[:, c1, :], t_ps[:])

        def b_s34(b, hs):
            # S^T_h = wk_ext_h^T T''_h (softmax axis lands on partitions)
            s = st[b]
            if "s_ps" not in s:
                s["s_ps"] = bpool(b).tile([48, H, 48], F32, tag=btag(b),
                                          name="s_ps")
            s_ps, t_sb = s["s_ps"], s["t_sb"]
            for h in hs:
                hsl = slice(48 * h, 48 * (h + 1))
                for kc1 in range(3):
                    nc.tensor.matmul(
                        s_ps[:, h, :], wk_sb[:, kc1, hsl], t_sb[:, kc1, hsl],
                        start=(kc1 == 0), stop=False)
                nc.tensor.matmul(s_ps[:, h, :], bk_row[:, hsl],
                                 s["t_row"][:, hsl], start=False, stop=True)

        def b_s5a(b):
            s = st[b]
            p_allT = sm.tile([48, H, 48], BF16, tag="p_allT", name="p_allT")
            nc.scalar.activation(
                p_allT[:], s.pop("s_ps")[:],
                mybir.ActivationFunctionType.Exp, bias=0.0, scale=SCALE)
            s["p_allT"] = p_allT
            bd = sm.tile([128, 3, C], BF16, tag="bd", name=f"bd{b}")
            nc.vector.memset(bd[:], 0.0)
            s["bd"] = bd

        def b_s5b(b):
            s = st[b]
            z_ps = bpool(b).tile([1, C], F32, tag=btag(b), name="z_ps")
            nc.tensor.matmul(z_ps[:], ones48[:], s["p_allT"][:],
                             start=True, stop=True)
            zr = sm.tile([1, C], F32, tag="zr", name="zr")
            nc.vector.reciprocal(zr[:], z_ps[:])
            s["zr"] = zr

        def b_s5c(b):
            s = st[b]
            p_allT = s["p_allT"]
            zb_ps = bpool(b).tile([48, H, 48], F32, tag=btag(b), name="zb_ps")
            nc.tensor.matmul(zb_ps[:], ones_r[:], s["zr"][:],
                             start=True, stop=True)
            attn_t = sm.tile([48, H, 48], BF16, tag="attn_t", name="attn_t")
            nc.vector.tensor_mul(attn_t[:], p_allT[:], zb_ps[:])
            s["attn_t"] = attn_t

        def b_s6(b):
            # scatter attn^T into bd with column order m = 8d+h: head h's
            # columns are the strided slice m = h::8 (so stage5' PSUM columns
            # land in 3 clean strided APs for the at writes).
            s = st[b]
            attn_t, bd = s["attn_t"], s["bd"]
            bdv = bd[:].rearrange("p kc (d h) -> p kc h d", h=H)
            for kc in range(3):
                bd_ps = bpool(b).tile([128, C], F32, tag=btag(b), name="bd_ps")
                hs_here = [(i, h) for i, (kc_i, h, off) in enumerate(SCATTER)
                           if kc_i == kc]
                for i, h in hs_here:
                    nc.tensor.matmul(
                        bd_ps[:, 48 * h:48 * h + 48],
                        sh_sb[:, i, :], attn_t[:, h, :],
                        start=True, stop=True)
                for n, (i, h) in enumerate(hs_here):
                    eng = nc.scalar if n % 2 == 0 else nc.vector
                    if eng is nc.scalar:
                        nc.scalar.activation(
                            bdv[:, kc, h, :], bd_ps[:, 48 * h:48 * h + 48],
                            mybir.ActivationFunctionType.Identity,
                            bias=0.0, scale=1.0)
                    else:
                        nc.vector.tensor_copy(
                            bdv[:, kc, h, :], bd_ps[:, 48 * h:48 * h + 48])

        def wav_fold(b):
            # Wav_ext = [Wv' | bv]^T bd  ([C+1, C]), cast fp8 (x SCL), into
            # the pair-packed layout stage5' consumes.  The bias row is
            # broadcast to all 128 partitions (matching the host's all-ones
            # 4th row of xt8) at value SCL*bvd/128.  The rhs uses a strided
            # view of bd so Wav's columns come out v-ordered (class-major,
            # v = (m%3)*128 + m//3): stage5's PSUM->at copies then become
            # contiguous on both sides.
            s = st[b]
            bd = s["bd"]
            bdv3 = bd[:].rearrange("p kc (a m3) -> p kc m3 a", m3=3)
            wav8 = w8.tile([128, 2, 2, WVP], F8, tag="wav8", name=f"wav8{b}")
            s["wav8"] = wav8
            for c2 in range(3):
                wv_ps = bpool(b).tile([128, C], F32, tag=btag(b), name="wv_ps")
                for kc in range(3):
                    nc.tensor.matmul(
                        wv_ps[:], wv2_sb[:, kc, c2 * 128:(c2 + 1) * 128],
                        bdv3[:, kc, :, :], start=(kc == 0), stop=(kc == 2))
                nc.scalar.activation(
                    wav8[:, c2 // 2, c2 % 2, 0:C], wv_ps[:],
                    mybir.ActivationFunctionType.Copy, bias=0.0, scale=SCL)
            bvd_ps = bpool(b).tile([1, C], F32, tag=btag(b), name="bvd_ps")
            for kc in range(3):
                nc.tensor.matmul(bvd_ps[:], bv2_sb[:, kc:kc + 1],
                                 bdv3[:, kc, :, :], start=(kc == 0), stop=(kc == 2))
            bvd_row = sm.tile([1, C], BF16, tag="bvdr", name="bvdr")
            nc.vector.tensor_copy(bvd_row[:], bvd_ps[:])
            bc_ps = bpool(b).tile([128, C], F32, tag=btag(b), name="bc_ps")
            nc.tensor.matmul(bc_ps[:], ones_row[:], bvd_row[:],
                             start=True, stop=True)
            nc.scalar.activation(
                wav8[:, 1, 1, 0:C], bc_ps[:],
                mybir.ActivationFunctionType.Copy, bias=0.0, scale=SCL / 128)

        # ---- stage5': at_c[tok, tc, v] = [X;1]^T Wav in fp8 DoubleRow ----
        # at_c is CHUNK-major: each chunk's PSUM is dumped contiguously
        # (v = (m%3)*128 + m//3 column order, from the bd strided view).
        # stage6 uses permuted output token blocks (tau-blocks, t' = 32u +
        # (11*tau)%32) so its stationary reads are contiguous at_c slices:
        # for pass jc, q = 3*((11*tau)%32) + jc -> lhsT = at_c[:, q%32,
        # (q//32)*128 : +128].  The host inverts the token permutation.
        def c_start(b):
            s = st[b]
            s["at"] = atp.tile([128, NCHUNK, C], BF16, tag="at",
                               name=f"at{b}")

        def c5_chunk(b, t):
            s = st[b]
            w, q = t // 4, t % 4
            xt, wav8 = xt_sb[b], s["wav8"]
            pool, tg = (ps_c, "psc") if t % 2 == 0 else (ps_v, "vps")
            at_ps = pool.tile([128, C], F32, tag=tg, name="at_ps")
            for p in range(2):
                nc.tensor.matmul(
                    at_ps[:],
                    xt[:, w, p, :, q * 128:(q + 1) * 128],
                    wav8[:, p, :, 0:C],
                    start=(p == 0), stop=(p == 1),
                    perf_mode=mybir.MatmulPerfMode.DoubleRow,
                )
            at_c = s["at"]
            nc.scalar.activation(
                at_c[:, t, 0:192], at_ps[:, 0:192],
                mybir.ActivationFunctionType.Identity, bias=0.0, scale=1.0)
            nc.vector.tensor_copy(at_c[:, t, 192:C], at_ps[:, 192:C])

        # ---- stage6: y = at-blocks @ wp + bp (bf16, tau-permuted rows) ----
        def c6_mms(b, tau):
            s = st[b]
            at_c = s["at"]
            y_ps = ps_g.tile([128, C], F32, tag="g", name="y_ps")
            rho = (11 * tau) % 32
            for jc in range(3):
                q = 3 * rho + jc
                nc.tensor.matmul(
                    y_ps[:],
                    at_c[:, q % 32, (q // 32) * 128:(q // 32 + 1) * 128],
                    wp_sb[:, jc, :],
                    start=(jc == 0), stop=(jc == 2))
            return y_ps

        def c_out(b, tau, blk, last=False):
            s = st[b]
            if s.get("y_blk") is None:
                s["y_sb"] = yo.tile([128, blk, C], BF16, tag="ysb", name="y_sb")
                s["y_blk"], s["y_lo"] = blk, tau
            y_ps = c6_mms(b, tau)
            nc.vector.tensor_add(s["y_sb"][:, tau - s["y_lo"], :], y_ps[:],
                                 bp_bc[:])
            if tau - s["y_lo"] == s["y_blk"] - 1:
                dst = outp.ap()[b].rearrange("t u c -> u t c")
                dst = dst[:, s["y_lo"]:tau + 1, :]
                if last:
                    nc.scalar.dma_start(dst[0:64], s["y_sb"][0:64])
                    nc.sync.dma_start(dst[64:128], s["y_sb"][64:128])
                else:
                    nc.sync.dma_start(dst, s["y_sb"][:])
                s["y_blk"] = None

        def c_out_last(b, tau):
            # single-block writebacks round-robined over four DMA queues so
            # the final data drains as it is produced instead of in a burst
            s = st[b]
            y_sb = yo.tile([128, 1, C], BF16, tag="ysb", name="y_sb")
            y_ps = c6_mms(b, tau)
            nc.vector.tensor_add(y_sb[:, 0, :], y_ps[:], bp_bc[:])
            dst = outp.ap()[b].rearrange("t u c -> u t c")
            dst = dst[:, tau:tau + 1, :]
            eng = nc.scalar if tau % 2 == 0 else nc.sync
            eng.dma_start(dst[0:64], y_sb[0:64])
            eng2 = nc.sync if tau % 2 == 0 else nc.scalar
            eng2.dma_start(dst[64:128], y_sb[64:128])

        # ---- schedule ------------------------------------------------------
        for w in range(NWIN):
            g_part(0, w)
            if w < NWIN - 1:
                keep_alive(3)

        b_s1_act(0)
        g_part(1, 0)
        keep_alive(2)
        b_s1_pe(0)
        g_part(1, 1)
        keep_alive(2)
        b_s2(0)
        g_part(1, 2)
        keep_alive(2)
        b_s34(0, range(0, 4))
        g_part(1, 3)
        keep_alive(2)
        b_s34(0, range(4, 8))
        b_s5a(0)
        g_part(1, 4)
        keep_alive(2)
        b_s5b(0)
        g_part(1, 5)
        keep_alive(2)
        b_s5c(0)
        g_part(1, 6)
        keep_alive(2)
        b_s6(0)
        g_part(1, 7)
        keep_alive(2)
        wav_fold(0)
        keep_alive(2)

        c_start(0)
        b1_stages = [lambda: b_s1_act(1), lambda: b_s1_pe(1),
                     lambda: b_s2(1),
                     lambda: b_s34(1, range(0, 4)),
                     lambda: (b_s34(1, range(4, 8)), b_s5a(1)),
                     lambda: b_s5b(1),
                     lambda: b_s5c(1), lambda: b_s6(1),
                     lambda: wav_fold(1)]
        for t in range(NCHUNK):
            c5_chunk(0, t)
            if t % 2 == 1 and t // 2 < len(b1_stages):
                b1_stages[t // 2]()

        c_start(1)
        for i in range(NCHUNK):
            c_out(0, i, 4)
            c5_chunk(1, i)
        for rw in range(0, 24):
            c_out(1, rw, 4)
        for rw in range(24, NCHUNK):
            c_out_last(1, rw)

    nc.compile()
    return nc


_CACHE = {}


def prepare_in_maps(x, conv_w, conv_b, wq, bq, wkv, bkv, wp, bp):
    import ml_dtypes

    bf16 = ml_dtypes.bfloat16
    f8 = ml_dtypes.float8_e4m3
    f32 = np.float32
    x = np.ascontiguousarray(x, dtype=f32)

    # fold the 1x1 conv into the projections (host-side weight prep)
    wk_w, wv_w = wkv[:C], wkv[C:]
    bk_b, bv_b = bkv[:C], bkv[C:]
    wqT = np.ascontiguousarray((wq @ conv_w).T, dtype=bf16)
    wkT = np.ascontiguousarray((wk_w @ conv_w).T, dtype=bf16)
    wv2 = np.ascontiguousarray(wv_w @ conv_w, dtype=bf16)    # [c, c2] no T
    wpT = np.ascontiguousarray(wp.T / SCL, dtype=bf16)       # pre-scaled
    bq_e = (bq + wq @ conv_b).astype(f32)
    bk_e = (bk_b + wk_w @ conv_b).astype(f32)
    bv_e = np.ascontiguousarray(bv_b + wv_w @ conv_b, dtype=bf16)
    bp_c = np.ascontiguousarray(bp, dtype=f32)

    xq = x.astype(f8)                                        # one quantization
    # Gram input: token-partition, per-chunk rows contiguous per partition
    xn = np.zeros((B, N, XNP), dtype=f8)
    xn[:, :, :C] = xq
    xn[:, :, C] = 1.0
    xn = np.ascontiguousarray(
        xn.reshape(B, NCHUNK, 128, XNP).transpose(0, 2, 1, 3))
    # stage5' input: channel-partition pair-packed [p, w, pass, pair, t]
    # rows: (c=p, c=128+p), (c=256+p, ones)
    xt = xq.transpose(0, 2, 1).reshape(B, 3, 128, NWIN, NW)  # [b, kc, p, w, t]
    xt8 = np.ones((B, 128, NWIN, 4, NW2), dtype=f8)
    xt8[:, :, :, 0:3, 0:NW] = xt.transpose(0, 2, 3, 1, 4)
    xt8[:, :, :, :, NW:] = 0.0
    xt8 = np.ascontiguousarray(xt8.reshape(B, 128, NWIN, 2, 2, NW2))

    bq_bf = np.ascontiguousarray(bq_e.reshape(1, C), dtype=bf16)
    bk_bf = np.ascontiguousarray(bk_e.reshape(1, C), dtype=bf16)
    bqn = np.ascontiguousarray((bq_e * N).reshape(1, C), dtype=f32)

    in_maps = []
    for c in range(N_CORES):
        in_maps.append({
            "xn": xn[c * BPC:(c + 1) * BPC],
            "xt8": xt8[c * BPC:(c + 1) * BPC],
            "wqT": wqT, "wkT": wkT, "wv2": wv2, "wpT": wpT,
            "bq": bq_bf, "bk": bk_bf, "bqN": bqn, "bv2": bv_e, "bp": bp_c,
        })

    return in_maps


def kernel(x, conv_w, conv_b, wq, bq, wkv, bkv, wp, bp):
    _install_ntff_hook()
    in_maps = prepare_in_maps(x, conv_w, conv_b, wq, bq, wkv, bkv, wp, bp)
    if "nc" not in _CACHE:
        _CACHE["nc"] = build()
    nc = _CACHE["nc"]
    res = run_bass_kernel_spmd(nc, in_maps, core_ids=list(range(N_CORES)))
    out = np.concatenate([res.results[c]["out"] for c in range(N_CORES)], axis=0)
    # invert the tau-block token permutation: row (tau, u) holds token
    # 32*u + (11*tau) % 32
    tau = np.arange(NCHUNK)[:, None]
    u = np.arange(128)[None, :]
    n_idx = (32 * u + (11 * tau) % 32).reshape(-1)
    y = np.empty((B, N, C), dtype=out.dtype)
    y[:, n_idx, :] = out.reshape(B, N, C)
    return y.astype(np.float32)


# revision 24
# speedup vs baseline: 1.0059x; 1.0059x over previous
"""Trainium2 Bass kernel for the channel-attention module (v2, restructured).

Reference computation (B=16, N=4096, C=384, H=8, D=48):
    x_in = x @ conv_w.T + conv_b                      # 1x1 conv == linear
    q    = (x_in @ wq.T + bq)  -> [B,H,D,N]
    k, v = (x_in @ wkv.T + bkv) -> 2x [B,H,D,N]
    attn = softmax((q * N**-0.5) @ k^T, axis=-1)      # [B,H,D,D] (over N!)
    out  = attn @ v                                   # [B,H,D,N]
    out  = out.transpose(0,2,1,3).reshape(B,N,C)      # verbatim torch layout
    y    = out @ wp.T + bp

Key restructure vs v1: the v-projection GEMM is eliminated.  Since
out = attn @ (Wv X + bv), fold the block-diagonal attention matrix into
the value weights FIRST:  Wav = [Wv | bv]^T bd  (a tiny [C+1, C]
product), then Z = Wav^T [X; 1] is ONE full-size GEMM instead of two
(v-projection + attn@v).  Z is computed in fp8 DoubleRow (measured: DR
streams at 1 col/cycle like bf16 but contracts 256 rows per pass, so
2 passes replace bf16's 3 -- a 1.5x win); X^T is staged fp8
pair-packed by the host; Wav is cast to fp8 on-chip (ACT cast is RN,
bit-exact with ml_dtypes).  The bv bias rides in the contraction as a
ones-row (host) x bvd-row (on-chip) rank-1 term.

The critical layout trick: stage6 consumes PERMUTED output token
blocks (tau-blocks, t' = 32u + (11*tau)%32, inverted on the host), so
at can stay CHUNK-major (each stage5' PSUM tile dumps contiguously)
while stage6's stationary reads are still contiguous single-stride
slices (the stationary AP allows only one free dimension).  Strided
ACT/DVE copies (64B-hop dst) measure ~5x slower than contiguous and
previously paced the whole kernel, idling the PE long enough to
re-trigger the HAM clock throttle (1.2 GHz) on top.

Per batch (2 per core, data-parallel over B across 8 cores):
  A) Gram G' = [X|1]^T[X|1] in fp8e4 DoubleRow (unchanged from v1),
  B) S-chain from G' (T''=G'wq_ext, S^T per head, exp/softmax) ->
     attn^T -> block-diag bd with column order m = 8d+h (strided ACT
     copies place head h at columns h::8),
  C) Wav fold (tiny GEMM + bias row) -> wav8 fp8,
  D) stage5': at[tok, m] = [X;1]^T Wav, 2 DoubleRow passes per
     128-token chunk; PSUM scattered into at[p, jc, r] (3r+jc = 32m+tc)
     so stage6 reads clean 128-column blocks,
  E) stage6: y = at-blocks @ wp(/256 pre-scaled) + bp, bf16 (fp8 here
     would break the 2e-2 error budget: each fp8 operand on the value
     path costs ~1.1e-2 rel err).

Schedule: batch pipelined -- Gram b1 runs between batch 0's S-chain
stages, stage5'(0) interleaves with S-chain(1), stage6(0) interleaves
with stage5'(1).  Writebacks stream on HWDGE queues; the final 8 blocks
go as split single chunks so the tail drains with the compute.
"""

import sys
import types
from contextlib import ExitStack

import numpy as np

import concourse.bass as bass
import concourse.tile as tile
from concourse import bacc, mybir
from concourse.bass_utils import run_bass_kernel_spmd
from concourse.masks import make_identity

B, N, C, H, D = 16, 4096, 384, 8, 48
N_CORES = 8
BPC = B // N_CORES          # batches per core
NW = 512                    # token window
NW2 = 528                   # xt8 padded row (avoid power-of-2 SBUF strides)
WVP = 400                   # wav8 padded row
NWIN = N // NW              # 8 windows
NCHUNK = N // 128           # 32 token chunks of 128
XNP = 400                   # xn row pad (>= C+1, 16B-aligned for dual-fp8 LDW)
SCALE = float(N) ** -0.5    # 1/64
SCL = 256.0                 # fp8 scale for Wav (wp is pre-divided on host)
F32 = mybir.dt.float32
BF16 = mybir.dt.bfloat16
F8 = mybir.dt.float8e4

# block-diag scatter: (kc, h, off) with off = 48h - 128kc; Sh[dj, c_p] = 1
# iff c_p == dj + off places head h's attn^T rows into bd tile kc.
SCATTER = [
    (0, 0, 0), (0, 1, 48), (0, 2, 96),
    (1, 2, -32), (1, 3, 16), (1, 4, 64), (1, 5, 112),
    (2, 5, -16), (2, 6, 32), (2, 7, 80),
]


def _install_ntff_hook():
    """The agent image's antenv lacks axon_hooks, so trn_boot's NTFF hook
    registration degrades silently and trace=True would crash.  Recreate the
    module and register the ctypes hook so profiling works."""
    try:
        import antenv

        if "antenv.axon_hooks" in sys.modules:
            return
        mod = types.ModuleType("antenv.axon_hooks")
        mod._hook = None
        mod.set_axon_ntff_profile_hook = lambda h: setattr(mod, "_hook", h)
        mod.get_axon_ntff_profile_hook = lambda: mod._hook
        sys.modules["antenv.axon_hooks"] = mod
        antenv.axon_hooks = mod
        from trn_agent_boot.trn_boot import _ntff_profile_via_ctypes

        mod.set_axon_ntff_profile_hook(
            _ntff_profile_via_ctypes("/opt/axon/libaxon_pjrt.so")
        )
    except Exception:
        pass


def build():
    nc = bacc.Bacc("TRN2", target_bir_lowering=False, debug=False,
                   num_devices=N_CORES)

    xn_p = nc.declare_dram_parameter("xn", [BPC, 128, NCHUNK, XNP], F8,
                                     isOutput=False)
    xt_p = nc.declare_dram_parameter("xt8", [BPC, 128, NWIN, 2, 2, NW2], F8,
                                     isOutput=False)
    wq_p = nc.declare_dram_parameter("wqT", [C, C], BF16, isOutput=False)
    wk_p = nc.declare_dram_parameter("wkT", [C, C], BF16, isOutput=False)
    wv_p = nc.declare_dram_parameter("wv2", [C, C], BF16, isOutput=False)
    wp_p = nc.declare_dram_parameter("wpT", [C, C], BF16, isOutput=False)
    bq_p = nc.declare_dram_parameter("bq", [1, C], BF16, isOutput=False)
    bk_p = nc.declare_dram_parameter("bk", [1, C], BF16, isOutput=False)
    bqn_p = nc.declare_dram_parameter("bqN", [1, C], F32, isOutput=False)
    bv_p = nc.declare_dram_parameter("bv2", [C], BF16, isOutput=False)
    bp_p = nc.declare_dram_parameter("bp", [C], F32, isOutput=False)
    outp = nc.declare_dram_parameter("out", [BPC, NCHUNK, 128, C], BF16,
                                    isOutput=True)
    scr_p = nc.declare_dram_parameter("scr", [128, 1], F32, isOutput=True)

    with tile.TileContext(nc) as tc, ExitStack() as ctx:
        const = ctx.enter_context(tc.tile_pool(name="const", bufs=1))
        xnr = ctx.enter_context(tc.tile_pool(name="xnr", bufs=2))
        xtp = ctx.enter_context(tc.tile_pool(name="xtp", bufs=2))
        atp = ctx.enter_context(tc.tile_pool(name="atp", bufs=2))
        sm = ctx.enter_context(tc.tile_pool(name="sm", bufs=2))
        w8 = ctx.enter_context(tc.tile_pool(name="w8", bufs=2))
        yo = ctx.enter_context(tc.tile_pool(name="yo", bufs=6))
        ps_g = ctx.enter_context(tc.tile_pool(name="ps_g", bufs=3, space="PSUM"))
        ps_c = ctx.enter_context(tc.tile_pool(name="ps_c", bufs=3, space="PSUM"))
        ps_v = ctx.enter_context(tc.tile_pool(name="ps_v", bufs=2, space="PSUM"))

        # ---- PE warmup tile first (gpsimd memset; no DMA dependency) ------
        warm = const.tile([128, NW], BF16)
        nc.gpsimd.memset(warm[:], 0.0)

        # ---- input streams -------------------------------------------------
        # Gram input (token-partition fp8), whole batches resident; the first
        # window of batch 0 goes first in small pieces so the Gram isn't
        # gated on the whole prefetch backlog.
        xn_sb = [xnr.tile([128, NCHUNK, XNP], F8, tag="xn", name=f"xn{b}")
                 for b in range(BPC)]
        # batch 0's Gram input split across both HWDGE queues, window-paired
        for lo in range(0, NCHUNK, 4):
            nc.sync.dma_start(xn_sb[0][:, lo:lo + 2, :],
                              xn_p.ap()[0][:, lo:lo + 2, :])
            nc.scalar.dma_start(xn_sb[0][:, lo + 2:lo + 4, :],
                                xn_p.ap()[0][:, lo + 2:lo + 4, :])
        for lo in range(0, NCHUNK, 8):
            nc.sync.dma_start(xn_sb[1][:, lo:lo + 8, :],
                              xn_p.ap()[1][:, lo:lo + 8, :])

        # weights on the ACT HWDGE queue (needed by the S-chain / Wav fold)
        def load_w(param, dtype):
            t = const.tile([128, 3, C], dtype, tag=f"w_{param.name}")
            nc.scalar.dma_start(
                t[:], param.ap().rearrange("(kc p) o -> p kc o", p=128))
            return t

        wq_sb = load_w(wq_p, BF16)
        wk_sb = load_w(wk_p, BF16)
        wv2_sb = load_w(wv_p, BF16)
        bq_row = const.tile([1, C], BF16)
        nc.scalar.dma_start(bq_row[:], bq_p.ap()[:, :])
        bk_row = const.tile([1, C], BF16)
        nc.scalar.dma_start(bk_row[:], bk_p.ap()[:, :])
        bqn_row = const.tile([1, C], F32)
        nc.scalar.dma_start(bqn_row[:], bqn_p.ap()[:, :])
        bv2_sb = const.tile([128, 3], BF16)
        nc.scalar.dma_start(bv2_sb[:],
                            bv_p.ap().rearrange("(kc p) -> p kc", p=128))

        # stage5' input (channel-partition pair-packed fp8), per batch
        # stage5' input: batch 0 on the ACT HWDGE queue, batch 1 on the
        # gpsimd software DGE so neither blocks the Gram streams.
        xt_sb = [xtp.tile([128, NWIN, 2, 2, NW2], F8, tag="xt", name=f"xt{b}")
                 for b in range(BPC)]
        for wlo in range(0, NWIN, 2):
            nc.scalar.dma_start(xt_sb[0][:, wlo:wlo + 2],
                                xt_p.ap()[0][:, wlo:wlo + 2])
        wp_sb = load_w(wp_p, BF16)
        bp_ap = bp_p.ap()
        bp_bc = const.tile([128, C], F32)
        nc.scalar.dma_start(bp_bc[:], bass.AP(
            tensor=bp_ap.tensor, offset=bp_ap.offset,
            ap=[[0, 128], *bp_ap.ap]))
        for wlo in range(0, NWIN, 4):
            nc.gpsimd.dma_start(xt_sb[1][:, wlo:wlo + 4],
                                xt_p.ap()[1][:, wlo:wlo + 4])

        # ---- small constants built on gpsimd (no DMA) ---------------------
        sh_sb = const.tile([48, len(SCATTER), 128], BF16, tag="sh")
        nc.gpsimd.memset(sh_sb[:], 0.0)
        for i, (kc, h, off) in enumerate(SCATTER):
            nc.gpsimd.affine_select(
                out=sh_sb[:, i, :], in_=sh_sb[:, i, :],
                compare_op=mybir.AluOpType.not_equal,
                fill=1.0, base=off, pattern=[[-1, 128]], channel_multiplier=1,
            )
        id128 = const.tile([128, 128], BF16)
        make_identity(nc, id128[:])
        ones48 = const.tile([48, 1], BF16)
        nc.gpsimd.memset(ones48[:], 1.0)
        ones_r = const.tile([1, 48], F32)
        nc.gpsimd.memset(ones_r[:], 1.0)
        ones_row = const.tile([1, 128], BF16)
        nc.gpsimd.memset(ones_row[:], 1.0)

        # warmup matmuls ramp the PE p-state while the first DMAs land
        warm_ps = ps_c.tile([128, NW], F32, tag="psc", name="warm_ps")
        for i in range(10):
            nc.tensor.matmul(warm_ps[:, 0:256], warm[:, 0:128], warm[:, 0:256],
                             start=(i == 0), stop=(i == 9))
        warm_sb = const.tile([128, 1], F32)
        nc.vector.tensor_copy(warm_sb[:], warm_ps[:, 0:1])
        nc.scalar.dma_start(scr_p.ap()[:, :], warm_sb[:])
        exp_warm = const.tile([1, 1], F32)
        nc.scalar.activation(exp_warm[:], warm_ps[0:1, 0:1],
                             mybir.ActivationFunctionType.Exp,
                             bias=0.0, scale=1.0)

        # ---- per-batch state ----------------------------------------------
        st = [dict() for _ in range(BPC)]

        def g_part(b, w):
            # Gram in fp8 DoubleRow: 2 token-chunks (256 contraction) per mm.
            s = st[b]
            if w == 0:
                s["g_ps"] = [ps_g.tile([128, C + 1], F32, tag="g",
                                       name=f"g{b}_{i}") for i in range(3)]
            xn = xn_sb[b]
            for sp in range(2):
                tp = 2 * w + sp
                ksl = slice(4 * w + 2 * sp, 4 * w + 2 * sp + 2)
                for oc in range(3):
                    nc.tensor.matmul(
                        s["g_ps"][oc][:],
                        xn[:, ksl, oc * 128:(oc + 1) * 128],
                        xn[:, ksl, 0:C + 1],
                        start=(tp == 0), stop=(tp == 2 * NWIN - 1),
                        perf_mode=mybir.MatmulPerfMode.DoubleRow,
                    )

        # ---- S-chain stages.  Batch 1's transient PSUM tiles use the Gram
        # pool (its banks are free by then); batch 0's use ps_c.
        def bpool(b):
            return ps_g if b == 1 else ps_c

        def btag(b):
            return "g" if b == 1 else "psc"

        def b_s1_act(b):
            s = st[b]
            g_sb = sm.tile([128, 3, C + 1], BF16, tag="g_sb", name=f"g_sb{b}")
            s["g_sb"] = g_sb
            for oc in range(2):
                nc.scalar.activation(
                    g_sb[:, oc, :], s["g_ps"][oc][:],
                    mybir.ActivationFunctionType.Identity, bias=0.0, scale=1.0)
            nc.vector.tensor_copy(g_sb[:, 2, :], s["g_ps"][2][:])

        def b_s1_pe(b):
            s = st[b]
            g_sb = s["g_sb"]
            # s^T row via identity matmul (s lives in G's ones column)
            st_ps = bpool(b).tile([1, C], F32, tag=btag(b), name="st_ps")
            for kc in range(3):
                nc.tensor.matmul(st_ps[:, kc * 128:(kc + 1) * 128],
                                 g_sb[:, kc, C:C + 1], id128[:],
                                 start=True, stop=True)
            st_row = sm.tile([1, C], BF16, tag="st_row", name="st_row")
            nc.vector.tensor_copy(st_row[:], st_ps[:])
            # T'' row C = wq^T s + N bq
            tr_ps = bpool(b).tile([1, C], F32, tag=btag(b), name="tr_ps")
            for kc in range(3):
                nc.tensor.matmul(tr_ps[:], g_sb[:, kc, C:C + 1],
                                 wq_sb[:, kc, :],
                                 start=(kc == 0), stop=(kc == 2))
            t_row = sm.tile([1, C], BF16, tag="t_row", name="t_row")
            nc.vector.tensor_add(t_row[:], tr_ps[:], bqn_row[:])
            s["st_row"], s["t_row"] = st_row, t_row

        def b_s2(b):
            # T'' = G' wq_ext, c1 blocks (G symmetry: lhsT from G rows)
            s = st[b]
            g_sb = s["g_sb"]
            t_sb = sm.tile([128, 3, C], BF16, tag="t_sb", name=f"t_sb{b}")
            s["t_sb"] = t_sb
            for c1 in range(3):
                t_ps = bpool(b).tile([128, C], F32, tag=btag(b), name="t_ps")
                for kc2 in range(3):
                    nc.tensor.matmul(
                        t_ps[:],
                        g_sb[:, kc2, c1 * 128:(c1 + 1) * 128],
                        wq_sb[:, kc2, :],
                        start=(kc2 == 0), stop=False)
                nc.tensor.matmul(
                    t_ps[:], s["st_row"][:, c1 * 128:(c1 + 1) * 128],
                    bq_row[:], start=False, stop=True)
                if c1 < 2:
                    nc.scalar.activation(
                        t_sb[:, c1, :], t_ps[:],
                        mybir.ActivationFunctionType.Identity,
                        bias=0.0, scale=1.0)
                else:
                    nc.vector.tensor_copy(t_sb[:, c1, :], t_ps[:])

        def b_s34(b, hs):
            # S^T_h = wk_ext_h^T T''_h (softmax axis lands on partitions)
            s = st[b]
            if "s_ps" not in s:
                s["s_ps"] = bpool(b).tile([48, H, 48], F32, tag=btag(b),
                                          name="s_ps")
            s_ps, t_sb = s["s_ps"], s["t_sb"]
            for h in hs:
                hsl = slice(48 * h, 48 * (h + 1))
                for kc1 in range(3):
                    nc.tensor.matmul(
                        s_ps[:, h, :], wk_sb[:, kc1, hsl], t_sb[:, kc1, hsl],
                        start=(kc1 == 0), stop=False)
                nc.tensor.matmul(s_ps[:, h, :], bk_row[:, hsl],
                                 s["t_row"][:, hsl], start=False, stop=True)

        def b_s5a(b):
            s = st[b]
            p_allT = sm.tile([48, H, 48], BF16, tag="p_allT", name="p_allT")
            nc.scalar.activation(
                p_allT[:], s.pop("s_ps")[:],
                mybir.ActivationFunctionType.Exp, bias=0.0, scale=SCALE)
            s["p_allT"] = p_allT
            bd = sm.tile([128, 3, C], BF16, tag="bd", name=f"bd{b}")
            nc.vector.memset(bd[:], 0.0)
            s["bd"] = bd

        def b_s5b(b):
            s = st[b]
            z_ps = bpool(b).tile([1, C], F32, tag=btag(b), name="z_ps")
            nc.tensor.matmul(z_ps[:], ones48[:], s["p_allT"][:],
                             start=True, stop=True)
            zr = sm.tile([1, C], F32, tag="zr", name="zr")
            nc.vector.reciprocal(zr[:], z_ps[:])
            s["zr"] = zr

        def b_s5c(b):
            s = st[b]
            p_allT = s["p_allT"]
            zb_ps = bpool(b).tile([48, H, 48], F32, tag=btag(b), name="zb_ps")
            nc.tensor.matmul(zb_ps[:], ones_r[:], s["zr"][:],
                             start=True, stop=True)
            attn_t = sm.tile([48, H, 48], BF16, tag="attn_t", name="attn_t")
            nc.vector.tensor_mul(attn_t[:], p_allT[:], zb_ps[:])
            s["attn_t"] = attn_t

        def b_s6(b):
            # scatter attn^T into bd with column order m = 8d+h: head h's
            # columns are the strided slice m = h::8 (so stage5' PSUM columns
            # land in 3 clean strided APs for the at writes).
            s = st[b]
            attn_t, bd = s["attn_t"], s["bd"]
            bdv = bd[:].rearrange("p kc (d h) -> p kc h d", h=H)
            for kc in range(3):
                bd_ps = bpool(b).tile([128, C], F32, tag=btag(b), name="bd_ps")
                hs_here = [(i, h) for i, (kc_i, h, off) in enumerate(SCATTER)
                           if kc_i == kc]
                for i, h in hs_here:
                    nc.tensor.matmul(
                        bd_ps[:, 48 * h:48 * h + 48],
                        sh_sb[:, i, :], attn_t[:, h, :],
                        start=True, stop=True)
                for n, (i, h) in enumerate(hs_here):
                    eng = nc.scalar if n % 2 == 0 else nc.vector
                    if eng is nc.scalar:
                        nc.scalar.activation(
                            bdv[:, kc, h, :], bd_ps[:, 48 * h:48 * h + 48],
                            mybir.ActivationFunctionType.Identity,
                            bias=0.0, scale=1.0)
                    else:
                        nc.vector.tensor_copy(
                            bdv[:, kc, h, :], bd_ps[:, 48 * h:48 * h + 48])

        def wav_fold(b):
            # Wav_ext = [Wv' | bv]^T bd  ([C+1, C]), cast fp8 (x SCL), into
            # the pair-packed layout stage5' consumes.  The bias row is
            # broadcast to all 128 partitions (matching the host's all-ones
            # 4th row of xt8) at value SCL*bvd/128.  The rhs uses a strided
            # view of bd so Wav's columns come out v-ordered (class-major,
            # v = (m%3)*128 + m//3): stage5's PSUM->at copies then become
            # contiguous on both sides.
            s = st[b]
            bd = s["bd"]
            bdv3 = bd[:].rearrange("p kc (a m3) -> p kc m3 a", m3=3)
            wav8 = w8.tile([128, 2, 2, WVP], F8, tag="wav8", name=f"wav8{b}")
            s["wav8"] = wav8
            for c2 in range(3):
                wv_ps = bpool(b).tile([128, C], F32, tag=btag(b), name="wv_ps")
                for kc in range(3):
                    nc.tensor.matmul(
                        wv_ps[:], wv2_sb[:, kc, c2 * 128:(c2 + 1) * 128],
                        bdv3[:, kc, :, :], start=(kc == 0), stop=(kc == 2))
                nc.scalar.activation(
                    wav8[:, c2 // 2, c2 % 2, 0:C], wv_ps[:],
                    mybir.ActivationFunctionType.Copy, bias=0.0, scale=SCL)
            bvd_ps = bpool(b).tile([1, C], F32, tag=btag(b), name="bvd_ps")
            for kc in range(3):
                nc.tensor.matmul(bvd_ps[:], bv2_sb[:, kc:kc + 1],
                                 bdv3[:, kc, :, :], start=(kc == 0), stop=(kc == 2))
            bvd_row = sm.tile([1, C], BF16, tag="bvdr", name="bvdr")
            nc.vector.tensor_copy(bvd_row[:], bvd_ps[:])
            bc_ps = bpool(b).tile([128, C], F32, tag=btag(b), name="bc_ps")
            nc.tensor.matmul(bc_ps[:], ones_row[:], bvd_row[:],
                             start=True, stop=True)
            nc.scalar.activation(
                wav8[:, 1, 1, 0:C], bc_ps[:],
                mybir.ActivationFunctionType.Copy, bias=0.0, scale=SCL / 128)

        # ---- stage5': at_c[tok, tc, v] = [X;1]^T Wav in fp8 DoubleRow ----
        # at_c is CHUNK-major: each chunk's PSUM is dumped contiguously
        # (v = (m%3)*128 + m//3 column order, from the bd strided view).
        # stage6 uses permuted output token blocks (tau-blocks, t' = 32u +
        # (11*tau)%32) so its stationary reads are contiguous at_c slices:
        # for pass jc, q = 3*((11*tau)%32) + jc -> lhsT = at_c[:, q%32,
        # (q//32)*128 : +128].  The host inverts the token permutation.
        def c_start(b):
            s = st[b]
            s["at"] = atp.tile([128, NCHUNK, C], BF16, tag="at",
                               name=f"at{b}")

        def c5_chunk(b, t):
            s = st[b]
            w, q = t // 4, t % 4
            xt, wav8 = xt_sb[b], s["wav8"]
            pool, tg = (ps_c, "psc") if t % 2 == 0 else (ps_v, "vps")
            at_ps = pool.tile([128, C], F32, tag=tg, name="at_ps")
            for p in range(2):
                nc.tensor.matmul(
                    at_ps[:],
                    xt[:, w, p, :, q * 128:(q + 1) * 128],
                    wav8[:, p, :, 0:C],
                    start=(p == 0), stop=(p == 1),
                    perf_mode=mybir.MatmulPerfMode.DoubleRow,
                )
            at_c = s["at"]
            nc.scalar.activation(
                at_c[:, t, 0:192], at_ps[:, 0:192],
                mybir.ActivationFunctionType.Identity, bias=0.0, scale=1.0)
            nc.vector.tensor_copy(at_c[:, t, 192:C], at_ps[:, 192:C])

        # ---- stage6: y = at-blocks @ wp + bp (bf16, tau-permuted rows) ----
        def c6_mms(b, tau):
            s = st[b]
            at_c = s["at"]
            y_ps = ps_g.tile([128, C], F32, tag="g", name="y_ps")
            rho = (11 * tau) % 32
            for jc in range(3):
                q = 3 * rho + jc
                nc.tensor.matmul(
                    y_ps[:],
                    at_c[:, q % 32, (q // 32) * 128:(q // 32 + 1) * 128],
                    wp_sb[:, jc, :],
                    start=(jc == 0), stop=(jc == 2))
            return y_ps

        def c_out(b, tau, blk, last=False):
            s = st[b]
            if s.get("y_blk") is None:
                s["y_sb"] = yo.tile([128, blk, C], BF16, tag="ysb", name="y_sb")
                s["y_blk"], s["y_lo"] = blk, tau
            y_ps = c6_mms(b, tau)
            nc.vector.tensor_add(s["y_sb"][:, tau - s["y_lo"], :], y_ps[:],
                                 bp_bc[:])
            if tau - s["y_lo"] == s["y_blk"] - 1:
                dst = outp.ap()[b].rearrange("t u c -> u t c")
                dst = dst[:, s["y_lo"]:tau + 1, :]
                nc.sync.dma_start(dst[0:64], s["y_sb"][0:64])
                nc.scalar.dma_start(dst[64:128], s["y_sb"][64:128])
                s["y_blk"] = None

        def c_out_last(b, tau):
            # single-block writebacks round-robined over four DMA queues so
            # the final data drains as it is produced instead of in a burst
            s = st[b]
            y_sb = yo.tile([128, 1, C], BF16, tag="ysb", name="y_sb")
            y_ps = c6_mms(b, tau)
            nc.vector.tensor_add(y_sb[:, 0, :], y_ps[:], bp_bc[:])
            dst = outp.ap()[b].rearrange("t u c -> u t c")
            dst = dst[:, tau:tau + 1, :]
            eng = nc.scalar if tau % 2 == 0 else nc.sync
            eng.dma_start(dst[0:64], y_sb[0:64])
            eng2 = nc.sync if tau % 2 == 0 else nc.scalar
            eng2.dma_start(dst[64:128], y_sb[64:128])

        # ---- schedule ------------------------------------------------------
        for w in range(NWIN):
            g_part(0, w)

        b_s1_act(0)
        g_part(1, 0)
        b_s1_pe(0)
        g_part(1, 1)
        b_s2(0)
        g_part(1, 2)
        b_s34(0, range(0, 4))
        g_part(1, 3)
        b_s34(0, range(4, 8))
        b_s5a(0)
        g_part(1, 4)
        b_s5b(0)
        g_part(1, 5)
        b_s5c(0)
        g_part(1, 6)
        b_s6(0)
        g_part(1, 7)
        wav_fold(0)

        c_start(0)
        b1_stages = [lambda: b_s1_act(1), lambda: b_s1_pe(1),
                     lambda: b_s2(1),
                     lambda: b_s34(1, range(0, 4)),
                     lambda: (b_s34(1, range(4, 8)), b_s5a(1)),
                     lambda: b_s5b(1),
                     lambda: b_s5c(1), lambda: b_s6(1),
                     lambda: wav_fold(1)]
        for t in range(NCHUNK):
            c5_chunk(0, t)
            if t % 2 == 1 and t // 2 < len(b1_stages):
                b1_stages[t // 2]()

        c_start(1)
        for i in range(NCHUNK):
            c_out(0, i, 4)
            c5_chunk(1, i)
        for rw in range(0, 24):
            c_out(1, rw, 4)
        for rw in range(24, NCHUNK):
            c_out_last(1, rw)

    nc.compile()
    return nc


_CACHE = {}


def prepare_in_maps(x, conv_w, conv_b, wq, bq, wkv, bkv, wp, bp):
    import ml_dtypes

    bf16 = ml_dtypes.bfloat16
    f8 = ml_dtypes.float8_e4m3
    f32 = np.float32
    x = np.ascontiguousarray(x, dtype=f32)

    # fold the 1x1 conv into the projections (host-side weight prep)
    wk_w, wv_w = wkv[:C], wkv[C:]
    bk_b, bv_b = bkv[:C], bkv[C:]
    wqT = np.ascontiguousarray((wq @ conv_w).T, dtype=bf16)
    wkT = np.ascontiguousarray((wk_w @ conv_w).T, dtype=bf16)
    wv2 = np.ascontiguousarray(wv_w @ conv_w, dtype=bf16)    # [c, c2] no T
    wpT = np.ascontiguousarray(wp.T / SCL, dtype=bf16)       # pre-scaled
    bq_e = (bq + wq @ conv_b).astype(f32)
    bk_e = (bk_b + wk_w @ conv_b).astype(f32)
    bv_e = np.ascontiguousarray(bv_b + wv_w @ conv_b, dtype=bf16)
    bp_c = np.ascontiguousarray(bp, dtype=f32)

    xq = x.astype(f8)                                        # one quantization
    # Gram input: token-partition, per-chunk rows contiguous per partition
    xn = np.zeros((B, N, XNP), dtype=f8)
    xn[:, :, :C] = xq
    xn[:, :, C] = 1.0
    xn = np.ascontiguousarray(
        xn.reshape(B, NCHUNK, 128, XNP).transpose(0, 2, 1, 3))
    # stage5' input: channel-partition pair-packed [p, w, pass, pair, t]
    # rows: (c=p, c=128+p), (c=256+p, ones)
    xt = xq.transpose(0, 2, 1).reshape(B, 3, 128, NWIN, NW)  # [b, kc, p, w, t]
    xt8 = np.ones((B, 128, NWIN, 4, NW2), dtype=f8)
    xt8[:, :, :, 0:3, 0:NW] = xt.transpose(0, 2, 3, 1, 4)
    xt8[:, :, :, :, NW:] = 0.0
    xt8 = np.ascontiguousarray(xt8.reshape(B, 128, NWIN, 2, 2, NW2))

    bq_bf = np.ascontiguousarray(bq_e.reshape(1, C), dtype=bf16)
    bk_bf = np.ascontiguousarray(bk_e.reshape(1, C), dtype=bf16)
    bqn = np.ascontiguousarray((bq_e * N).reshape(1, C), dtype=f32)

    in_maps = []
    for c in range(N_CORES):
        in_maps.append({
            "xn": xn[c * BPC:(c + 1) * BPC],
            "xt8": xt8[c * BPC:(c + 1) * BPC],
            "wqT": wqT, "wkT": wkT, "wv2": wv2, "wpT": wpT,
            "bq": bq_bf, "bk": bk_bf, "bqN": bqn, "bv2": bv_e, "bp": bp_c,
        })

    return in_maps


def kernel(x, conv_w, conv_b, wq, bq, wkv, bkv, wp, bp):
    _install_ntff_hook()
    in_maps = prepare_in_maps(x, conv_w, conv_b, wq, bq, wkv, bkv, wp, bp)
    if "nc" not in _CACHE:
        _CACHE["nc"] = build()
    nc = _CACHE["nc"]
    res = run_bass_kernel_spmd(nc, in_maps, core_ids=list(range(N_CORES)))
    out = np.concatenate([res.results[c]["out"] for c in range(N_CORES)], axis=0)
    # invert the tau-block token permutation: row (tau, u) holds token
    # 32*u + (11*tau) % 32
    tau = np.arange(NCHUNK)[:, None]
    u = np.arange(128)[None, :]
    n_idx = (32 * u + (11 * tau) % 32).reshape(-1)
    y = np.empty((B, N, C), dtype=out.dtype)
    y[:, n_idx, :] = out.reshape(B, N, C)
    return y.astype(np.float32)
